# revision 21
# baseline (speedup 1.0000x reference)
import sys

sys.path.insert(0, "/opt/trn_rl_repo")

import ml_dtypes
import numpy as np

import concourse.bass as bass
import concourse.tile as tile
from concourse import bacc, mybir
from concourse.bass_utils import run_bass_kernel_spmd

# Put the table holding {exp, square, copy} first so the greedy
# act-table-load inserter needs only the sqrt<->exp switches.
_orig_get_tables = bacc.get_activation_tables


def _reordered_tables(arch):
    t = dict(_orig_get_tables(arch))
    pri = "exp_and_others"
    if pri in t:
        return {pri: t[pri], **{k: v for k, v in t.items() if k != pri}}
    return t


bacc.get_activation_tables = _reordered_tables

F32 = mybir.dt.float32
F32R = mybir.dt.float32r
BF16 = mybir.dt.bfloat16
AF = mybir.ActivationFunctionType

BATCH = 2
SEQ = 2048
D = 1024
NHEADS = 16
DK = 64
HPC = 4          # heads per core
NCORES = 8
THETA = 10000.0
EPS = 1e-8
NEG = -30000.0
CHUNK = 512
NCH = SEQ // CHUNK   # 4 chunks of queries
NBLK = SEQ // 128    # 16 key blocks
RHO = [0, 2, 1, 3]   # head permutation of pA/pB row blocks (self-inverse)


def _build_nc():
    nc = bacc.Bacc("TRN2", target_bir_lowering=False)
    XT = nc.declare_dram_parameter("XT", [128, 8, SEQ], BF16, isOutput=False)
    WT = nc.declare_dram_parameter("WT", [128, 8, 768], BF16, isOutput=False)
    COS = nc.declare_dram_parameter("COS", [128, SEQ], BF16, isOutput=False)
    SIN = nc.declare_dram_parameter("SIN", [128, SEQ], BF16, isOutput=False)
    WOT = nc.declare_dram_parameter("WOT", [128, 2, D], BF16, isOutput=False)
    MASKD = nc.declare_dram_parameter("MASKD", [128, 128], BF16, isOutput=False)
    INDT = nc.declare_dram_parameter("INDT", [128, 4], BF16, isOutput=False)
    I2Q = nc.declare_dram_parameter("I2Q", [36, 128], F32, isOutput=False)
    I2K = nc.declare_dram_parameter("I2K", [36, 128], F32, isOutput=False)
    ID = nc.declare_dram_parameter("ID", [128, 128], BF16, isOutput=False)
    OUT = nc.declare_dram_parameter("OUT", [SEQ, D], F32, isOutput=True)

    with tile.TileContext(nc) as tc:
        with (
            nc.allow_low_precision(reason="bf16 matmuls validated at 1e-2 rel err"),
            tc.tile_pool(name="cst", bufs=1) as cst,
            tc.tile_pool(name="xtp", bufs=4) as xtp,
            tc.tile_pool(name="tmp", bufs=16) as tmp,
            tc.tile_pool(name="expp", bufs=4) as expp,
            tc.tile_pool(name="bcp", bufs=4) as bcp,
            tc.tile_pool(name="ocp", bufs=4) as ocp,
            tc.tile_pool(name="ps2", bufs=2, space="PSUM") as ps2,
            tc.tile_pool(name="ps1", bufs=2, space="PSUM") as ps1,
            tc.tile_pool(name="psn", bufs=1, space="PSUM") as psn,
            tc.tile_pool(name="psa", bufs=1, space="PSUM") as psa,
        ):
            wt_sb = cst.tile([128, 8, 768], BF16, tag="wt")
            cos_sb = cst.tile([128, SEQ], BF16, tag="cos")
            sin_sb = cst.tile([128, SEQ], BF16, tag="sin")
            wot_sb = cst.tile([128, 2, D], BF16, tag="wot")
            maskd_sb = cst.tile([128, 128], BF16, tag="maskd")
            indt_sb = cst.tile([128, 4], BF16, tag="indt")
            i2q_sb = cst.tile([36, 128], F32, tag="i2q")
            i2k_sb = cst.tile([36, 128], F32, tag="i2k")
            id_sb = cst.tile([128, 128], BF16, tag="id")
            q_sb = cst.tile([128, 2, SEQ], BF16, tag="q")
            k_sb = cst.tile([128, 2, SEQ], BF16, tag="k")
            v_sb = cst.tile([128, NBLK, HPC, 65], BF16, tag="v")
            ot_sb = cst.tile([128, 2, SEQ], BF16, tag="ot")

            def emit_xt(c):
                c0 = c * CHUNK
                xt_t = xtp.tile([128, 8, CHUNK], BF16, tag="xt", name=f"xt_{c}")
                nc.sync.dma_start(out=xt_t[:], in_=XT[:, :, c0:c0 + CHUNK])
                return xt_t

            # startup DMAs: first projection is k (cols 256:512), so load
            # those weight columns first
            xt0 = xtp.tile([128, 8, CHUNK], BF16, tag="xt", name="xt_0")
            nc.sync.dma_start(out=xt0[:, 0:4], in_=XT[:, 0:4, 0:CHUNK])
            nc.sync.dma_start(out=wt_sb[:, :, 256:512], in_=WT[:, :, 256:512])
            nc.sync.dma_start(out=xt0[:, 4:8], in_=XT[:, 4:8, 0:CHUNK])
            nc.sync.dma_start(out=wt_sb[:, :, 0:256], in_=WT[:, :, 0:256])
            nc.sync.dma_start(out=indt_sb[:], in_=INDT[:])
            nc.sync.dma_start(out=i2q_sb[:].bitcast(F32R), in_=I2Q[:].bitcast(F32R))
            nc.sync.dma_start(out=i2k_sb[:].bitcast(F32R), in_=I2K[:].bitcast(F32R))
            nc.sync.dma_start(out=cos_sb[:], in_=COS[:])
            nc.sync.dma_start(out=sin_sb[:], in_=SIN[:])

            # ones columns for the denominator trick (data cols overwritten)
            nc.vector.memset(v_sb[:], 1.0)

            def emit_proj_qk2(c, xt_t):
                # k and q projections with a SHARED [8,512] n2 tile so ONE
                # Sqrt instruction serves both: 2 act-table loads per chunk
                # instead of 4 (the scheduler cannot split one instruction).
                c0 = c * CHUNK
                n2t = psn.tile([36, CHUNK], F32, tag="nw", name=f"n2_{c}")
                nc.vector.memset(n2t[:], 1.0)   # define rows the sqrt reads
                ab = {}
                for qk in (1, 0):         # k rows 0:4, q rows 32:36
                    qoff = 256 * qk
                    ppa = ps1.tile([128, CHUNK], F32, tag="pp",
                                   name=f"ppa_{qk}_{c}")
                    ppb = ps1.tile([128, CHUNK], F32, tag="pp",
                                   name=f"ppb_{qk}_{c}")
                    pA = ppa[:]
                    pB = ppb[:]
                    for di in range(8):
                        nc.tensor.matmul(
                            pA, lhsT=wt_sb[:, di, qoff:qoff + 128],
                            rhs=xt_t[:, di],
                            start=(di == 0), stop=(di == 7),
                        )
                    for di in range(8):
                        nc.tensor.matmul(
                            pB, lhsT=wt_sb[:, di, qoff + 128:qoff + 256],
                            rhs=xt_t[:, di],
                            start=(di == 0), stop=(di == 7),
                        )
                    aS = tmp.tile([128, CHUNK], BF16, tag="t",
                                  name=f"aS_{qk}_{c}")
                    nc.vector.tensor_copy(aS[:], pA)
                    bS = tmp.tile([128, CHUNK], BF16, tag="t",
                                  name=f"bS_{qk}_{c}")
                    nc.vector.tensor_copy(bS[:], pB)
                    sqA = tmp.tile([128, CHUNK], BF16, tag="t",
                                   name=f"sqA_{qk}_{c}")
                    nc.scalar.activation(sqA[:], pA, AF.Square)
                    sqB = tmp.tile([128, CHUNK], BF16, tag="t",
                                   name=f"sqB_{qk}_{c}")
                    nc.scalar.activation(sqB[:], pB, AF.Square)
                    n2 = n2t[32 * (1 - qk):32 * (1 - qk) + 4, :]
                    nc.tensor.matmul(n2, lhsT=indt_sb[:], rhs=sqA[:],
                                     start=True, stop=False)
                    nc.tensor.matmul(n2, lhsT=indt_sb[:], rhs=sqB[:],
                                     start=False, stop=True)
                    ab[qk] = (aS, bS)

                nrm = tmp.tile([36, CHUNK], F32, tag="n", name=f"nrm_{c}")
                nc.scalar.sqrt(nrm[:], n2t[:])
                nrmr = tmp.tile([36, CHUNK], F32, tag="n", name=f"nrmr_{c}")
                nc.vector.reciprocal(nrmr[:].bitcast(F32R), nrm[:])

                for qk in (1, 0):
                    dst = q_sb if qk == 0 else k_sb
                    i2 = i2q_sb if qk == 0 else i2k_sb
                    aS, bS = ab[qk]
                    lnt = psn.tile([128, CHUNK], F32, tag="nw",
                                   name=f"rbp_{qk}_{c}")
                    rbp = lnt[:]
                    nc.tensor.matmul(
                        rbp, lhsT=i2[:].bitcast(F32R),
                        rhs=nrmr[:].bitcast(F32R), start=True, stop=True,
                    )
                    rb = tmp.tile([128, CHUNK], BF16, tag="t",
                                  name=f"rb_{qk}_{c}")
                    nc.vector.tensor_copy(rb[:], rbp)

                    cs = cos_sb[:, c0:c0 + CHUNK]
                    sn = sin_sb[:, c0:c0 + CHUNK]
                    rbc = tmp.tile([128, CHUNK], BF16, tag="t",
                                   name=f"rbc_{qk}_{c}")
                    nc.vector.tensor_mul(rbc[:], rb[:], cs)
                    rbs = tmp.tile([128, CHUNK], BF16, tag="t",
                                   name=f"rbs_{qk}_{c}")
                    nc.vector.tensor_mul(rbs[:], rb[:], sn)

                    tac = tmp.tile([128, CHUNK], BF16, tag="t",
                                   name=f"tac_{qk}_{c}")
                    nc.vector.tensor_mul(tac[:], aS[:], rbc[:])
                    tas = tmp.tile([128, CHUNK], BF16, tag="t",
                                   name=f"tas_{qk}_{c}")
                    nc.vector.tensor_mul(tas[:], aS[:], rbs[:])
                    tbc = tmp.tile([128, CHUNK], BF16, tag="t",
                                   name=f"tbc_{qk}_{c}")
                    nc.vector.tensor_mul(tbc[:], bS[:], rbc[:])
                    tbs = tmp.tile([128, CHUNK], BF16, tag="t",
                                   name=f"tbs_{qk}_{c}")
                    nc.vector.tensor_mul(tbs[:], bS[:], rbs[:])

                    # rotate+scatter into q_sb/k_sb. Row blocks are
                    # host-permuted by RHO so the gpsimd-assigned ops keep
                    # out partitions == in partitions.
                    for h in range(HPC):
                        po = (h % 2) * 64
                        ti = h // 2
                        hs = 32 * RHO[h]
                        eng_s = nc.gpsimd if h in (0, 1) else nc.vector
                        eng_a = nc.gpsimd if h in (2, 3) else nc.vector
                        eng_s.tensor_sub(
                            dst[po:po + 32, ti, c0:c0 + CHUNK],
                            tac[hs:hs + 32, :], tbs[hs:hs + 32, :])
                        eng_a.tensor_add(
                            dst[po + 32:po + 64, ti, c0:c0 + CHUNK],
                            tas[hs:hs + 32, :], tbc[hs:hs + 32, :])

            def emit_proj_v(c, half, xt_t):
                vt = ps1.tile([128, CHUNK], F32, tag="pp", name=f"vt_{half}_{c}")
                for bl in range(2):
                    bb = 2 * half + bl
                    nb = 4 * c + bb
                    vps = vt[:, 256 * bl:256 * bl + 256]
                    for di in range(8):
                        nc.tensor.matmul(
                            vps,
                            lhsT=xt_t[:, di, bb * 128:bb * 128 + 128],
                            rhs=wt_sb[:, di, 512:768],
                            start=(di == 0), stop=(di == 7),
                        )
                    nc.vector.tensor_copy(
                        v_sb[:, nb, :, 0:64], vt[:, 256 * bl:256 * bl + 256])

            def emit_attn_head(c, h):
                c0 = c * CHUNK
                po = (h % 2) * 64
                ti = h // 2
                av = psa.tile([65, CHUNK], F32, tag="av", name=f"av_{h}_{c}")
                npairs = 2 * c + 2   # 2c off-diag pairs + 2 diag pairs

                def issue_pair(p):
                    sct = ps2.tile([128, 1024], F32, tag="sc",
                                   name=f"sc_{h}_{c}_{p}")
                    if p < 2 * c:      # off-diagonal: full width
                        for half in range(2):
                            jb = 2 * p + half
                            nc.tensor.matmul(
                                sct[:, 512 * half:512 * half + 512],
                                lhsT=k_sb[po:po + 64, ti, jb * 128:jb * 128 + 128],
                                rhs=q_sb[po:po + 64, ti, c0:c0 + CHUNK],
                                start=True, stop=True,
                            )
                    else:              # diagonal pair: restricted + 128-col mask
                        dp = p - 2 * c
                        for half in range(2):
                            s = 2 * dp + half
                            jb = 4 * c + s
                            r0 = 512 * half + 128 * s
                            nc.tensor.matmul(
                                sct[:, r0:512 * half + 512],
                                lhsT=k_sb[po:po + 64, ti, jb * 128:jb * 128 + 128],
                                rhs=q_sb[po:po + 64, ti, c0 + 128 * s:c0 + CHUNK],
                                start=True, stop=False,
                            )
                            nc.tensor.matmul(
                                sct[:, r0:r0 + 128], lhsT=id_sb[:],
                                rhs=maskd_sb[:], start=False, stop=True,
                            )
                    return sct

                def drain_pair(p, sct, first, last):
                    ex = expp.tile([128, 1024], BF16, tag="ex",
                                   name=f"ex_{h}_{c}_{p}")
                    if p < 2 * c:
                        nc.scalar.activation(ex[:], sct[:], AF.Exp)
                        for half in range(2):
                            jb = 2 * p + half
                            nc.tensor.matmul(
                                av, lhsT=v_sb[:, jb, h, :],
                                rhs=ex[:, 512 * half:512 * half + 512],
                                start=(first and half == 0), stop=False,
                            )
                    else:
                        dp = p - 2 * c
                        for half in range(2):
                            s = 2 * dp + half
                            jb = 4 * c + s
                            r0 = 512 * half + 128 * s
                            nc.scalar.activation(
                                ex[:, r0:512 * half + 512],
                                sct[:, r0:512 * half + 512], AF.Exp)
                            nc.tensor.matmul(
                                av[:, 128 * s:CHUNK],
                                lhsT=v_sb[:, jb, h, :],
                                rhs=ex[:, r0:512 * half + 512],
                                start=(first and half == 0),
                                stop=(last and half == 1),
                            )

                # stagger: issue scores for pair p+1 before draining pair p
                cur = issue_pair(0)
                for p in range(npairs):
                    nxt = issue_pair(p + 1) if p + 1 < npairs else None
                    drain_pair(p, cur, p == 0, p == npairs - 1)
                    cur = nxt

                srec = bcp.tile([1, CHUNK], F32, tag="srec", name=f"srec_{h}_{c}")
                nc.vector.reciprocal(srec[:].bitcast(F32R), av[64:65, :])
                rb2 = bcp.tile([64, CHUNK], F32, tag="rb2", name=f"rb2_{h}_{c}")
                nc.gpsimd.partition_broadcast(rb2[:], srec[:])
                nc.vector.tensor_mul(
                    ot_sb[po:po + 64, ti, c0:c0 + CHUNK], av[0:64, :], rb2[:])

            def emit_outproj(c, pool="nw"):
                for bb in range(4):
                    nb = 4 * c + bb
                    if pool == "sc":
                        # attention is drained by now; use the score pairs
                        wop = ps2.tile([128, 1024], F32, tag="sc",
                                       name=f"wo_{nb}")
                        for oc in range(2):
                            for ti in range(2):
                                nc.tensor.matmul(
                                    wop[:, 512 * oc:512 * oc + 512],
                                    lhsT=ot_sb[:, ti, nb * 128:nb * 128 + 128],
                                    rhs=wot_sb[:, ti, 512 * oc:512 * oc + 512],
                                    start=(ti == 0), stop=(ti == 1),
                                )
                        ob = ocp.tile([128, 1024], F32, tag="ob2",
                                      name=f"ob_{nb}")
                        nc.vector.tensor_copy(ob[:], wop[:])
                        nc.sync.dma_start(
                            out=OUT[nb * 128:nb * 128 + 128, :], in_=ob[:])
                        continue
                    for oc in range(2):
                        wo = psn.tile([128, CHUNK], F32, tag="nw",
                                      name=f"wo_{nb}_{oc}")
                        for ti in range(2):
                            nc.tensor.matmul(
                                wo,
                                lhsT=ot_sb[:, ti, nb * 128:nb * 128 + 128],
                                rhs=wot_sb[:, ti, 512 * oc:512 * oc + 512],
                                start=(ti == 0), stop=(ti == 1),
                            )
                        ob = ocp.tile([128, CHUNK], F32, tag="ob",
                                      name=f"ob_{nb}_{oc}")
                        nc.vector.tensor_copy(ob[:], wo[:])
                        nc.sync.dma_start(
                            out=OUT[nb * 128:nb * 128 + 128,
                                    512 * oc:512 * oc + 512],
                            in_=ob[:])

            # Dense-projection schedule: project k/v/q for chunk 0 first so
            # attention can start, then push remaining projections early so
            # the big attn(3) tail overlaps outproj only.
            # k and q projections are emitted back-to-back so their Sqrt ops
            # are adjacent in ACT program order: Square lives in the sqrt
            # table too, so each chunk needs only 2 act-table loads.
            emit_proj_qk2(0, xt0)         # k(0)+q(0)
            nc.sync.dma_start(out=wt_sb[:, :, 512:768], in_=WT[:, :, 512:768])
            nc.sync.dma_start(out=id_sb[:], in_=ID[:])
            nc.sync.dma_start(out=maskd_sb[:], in_=MASKD[:])
            emit_proj_v(0, 0, xt0)
            emit_proj_v(0, 1, xt0)
            nc.sync.dma_start(out=wot_sb[:], in_=WOT[:])

            xts = {0: xt0}
            for c in range(1, NCH):
                xts[c] = emit_xt(c)

            # row 1: attn(0) + proj chunk 1
            emit_attn_head(0, 0)
            emit_proj_qk2(1, xts[1])      # k(1)+q(1)
            emit_attn_head(0, 1)
            emit_proj_v(1, 0, xts[1])
            emit_attn_head(0, 2)
            emit_proj_v(1, 1, xts[1])
            emit_attn_head(0, 3)

            # row 2: attn(1) + proj chunk 2
            emit_attn_head(1, 0)
            emit_proj_qk2(2, xts[2])      # k(2)+q(2)
            emit_attn_head(1, 1)
            emit_proj_v(2, 0, xts[2])
            emit_attn_head(1, 2)
            emit_proj_v(2, 1, xts[2])
            emit_attn_head(1, 3)
            emit_outproj(0)

            # row 3: attn(2) + proj chunk 3 + outproj(0,1)
            emit_attn_head(2, 0)
            emit_proj_qk2(3, xts[3])      # k(3)+q(3)
            emit_attn_head(2, 1)
            emit_proj_v(3, 0, xts[3])
            emit_attn_head(2, 2)
            emit_proj_v(3, 1, xts[3])
            emit_attn_head(2, 3)
            emit_outproj(1)

            # row 4: attn(3) tail + remaining outprojs
            emit_attn_head(3, 0)
            emit_attn_head(3, 1)
            emit_outproj(2)
            emit_attn_head(3, 2)
            emit_attn_head(3, 3)
            emit_outproj(3, pool="sc")
    return nc


_NC = None


def _get_nc():
    global _NC
    if _NC is None:
        _NC = _build_nc()
        _NC.finalize()
    return _NC


def _shared_tables(token_positions):
    freqs = np.arange(0, DK, 2, dtype=np.float64)
    inv_theta = THETA ** (-freqs / DK)                      # [32]
    pos = token_positions.astype(np.float64)
    ang = inv_theta[:, None] * pos[None, :]                 # [32, SEQ]
    cos_t = np.ascontiguousarray(
        np.tile(np.cos(ang), (4, 1))).astype(ml_dtypes.bfloat16)
    sin_t = np.ascontiguousarray(
        np.tile(np.sin(ang), (4, 1))).astype(ml_dtypes.bfloat16)

    p_i = np.arange(128)[:, None]
    t_i = np.arange(128)[None, :]
    maskd = np.where(t_i >= p_i, 0.0, NEG).astype(ml_dtypes.bfloat16)

    indt = np.zeros((128, 4), dtype=np.float32)
    for j in range(4):
        indt[32 * j:32 * j + 32, j] = 1.0
    i2k = np.ascontiguousarray(indt.T)
    idm = np.eye(128, dtype=ml_dtypes.bfloat16)
    return cos_t, sin_t, maskd, indt.astype(ml_dtypes.bfloat16), i2k, idm


def _core_inputs(c, x, W_QKV, W_O, qk_scale, shared):
    cos_t, sin_t, maskd, indt, i2k, idm = shared
    b = c // 4
    a = c % 4
    heads = [4 * a + i for i in range(HPC)]
    pheads = [heads[j] for j in RHO]   # row-block head order (see RHO)

    qA = [64 * h + 2 * t for h in pheads for t in range(32)]
    qB = [64 * h + 2 * t + 1 for h in pheads for t in range(32)]
    kA = [1024 + r for r in qA]
    kB = [1024 + r for r in qB]
    vr = [2048 + 64 * h + j for h in heads for j in range(DK)]
    rows = qA + qB + kA + kB + vr
    wt = np.ascontiguousarray(
        W_QKV[rows, :].T.reshape(8, 128, 768).transpose(1, 0, 2)
    ).astype(ml_dtypes.bfloat16)

    vcols = [64 * h + j for h in heads for j in range(DK)]
    wot = np.ascontiguousarray(
        W_O[:, vcols].T.reshape(2, 128, D).transpose(1, 0, 2)
    ).astype(ml_dtypes.bfloat16)

    xt = np.ascontiguousarray(
        x[b].T.reshape(8, 128, SEQ).transpose(1, 0, 2)
    ).astype(ml_dtypes.bfloat16)

    i2q = np.zeros((36, 128), dtype=np.float32)
    i2k8 = np.zeros((36, 128), dtype=np.float32)
    for j in range(4):
        i2q[32 + j, 32 * j:32 * j + 32] = np.float32(qk_scale[pheads[j]])
        i2k8[j, 32 * j:32 * j + 32] = 1.0

    return {
        "XT": xt, "WT": wt, "COS": cos_t, "SIN": sin_t, "WOT": wot,
        "MASKD": maskd, "INDT": indt, "I2Q": i2q, "I2K": i2k8, "ID": idm,
    }


def _run(inputs, trace=False):
    x = np.asarray(inputs["x"], dtype=np.float32)
    token_positions = np.asarray(inputs["token_positions"])
    W_QKV = np.asarray(inputs["W_QKV"], dtype=np.float32)
    W_O = np.asarray(inputs["W_O"], dtype=np.float32)
    qk_scale = np.asarray(inputs["qk_scale"], dtype=np.float32)

    shared = _shared_tables(token_positions)
    nc = _get_nc()
    in_maps = [_core_inputs(c, x, W_QKV, W_O, qk_scale, shared)
               for c in range(NCORES)]
    core_ids = list(range(NCORES))
    kw = {}
    if trace:
        kw = dict(trace=True, trace_cores=core_ids)
    res = run_bass_kernel_spmd(nc, in_maps, core_ids, **kw)
    parts = [np.asarray(r["OUT"], dtype=np.float32) for r in res.results]
    out = np.stack([
        parts[0] + parts[1] + parts[2] + parts[3],
        parts[4] + parts[5] + parts[6] + parts[7],
    ]).astype(np.float32)
    return out, getattr(res, "exec_time_ns", None)


def kernel(**inputs):
    return _run(inputs, trace=False)[0]


def estimate_time_ns():
    from concourse.timeline_sim import TimelineSim
    ts = TimelineSim(_get_nc(), trace=False, no_exec=True)
    return ts.simulate()


def kernel_timed(**inputs):
    out, _ = _run(inputs, trace=False)
    return out, estimate_time_ns()
